# revision 8
# baseline (speedup 1.0000x reference)
"""Trainium2 Bass kernel for the 2-relation GIN GNN layer (final).

Design highlights (each validated against neuron-profile traces):
  - edges packed per destination window as fp8 identity tiles (slot
    partition = dst) + variable one-hot overflow tiles baked from the
    graph (max over cores so the SPMD program is shared); self-edges
    folded into the same slot stream; scatter-sum via DoubleRow fp8
    matmuls against a resident identity pair / on-device one-hot S.
  - phase A software-pipelined: h1 matmuls run one window-pair behind
    the aggregation so the PE never waits on the ACT psum->sbuf copy.
  - BN batch stats via ACT copy-accumulate + DVE square-accumulate;
    ONE AllReduce at the end; a dummy warm-up collective at kernel
    start pays the ncfw entry cost, and rstd = exp(-0.5*ln(var)) keeps
    the scalar engine on a single activation table (no table reloads).
  - xT (feature-major x, needed only in phase C) is loaded inside the
    allreduce shadow: the stream DMA then owns all HBM bandwidth in
    phase A, and the collective gap's idle DMA does useful work.
  - phase C: gate logits via composed CPU-side weights; hbn/z/e
    emitted one super-block (2 window-pairs) ahead of the nm matmuls;
    one merged FD=512 rank-1 bias matmul per window-pair; cumsum via
    triangular-ones matmul; normalize/combine on DVE.
"""

import numpy as np
import ml_dtypes

import concourse.bass as bass
import concourse.mybir as mybir
import concourse.tile as tile
from concourse import bacc
from concourse.bass_utils import run_bass_kernel_spmd

F32 = mybir.dt.float32
BF16 = mybir.dt.bfloat16
FP8 = mybir.dt.float8e4
AX = mybir.AxisListType
OP = mybir.AluOpType
ACT = mybir.ActivationFunctionType
PM = mybir.MatmulPerfMode

BF = ml_dtypes.bfloat16
F8 = ml_dtypes.float8_e4m3

N_GLOBAL = 100000
E_GLOBAL = 1600000
CORES = 8
KI = 8

# column layout of the "vecs" [128, 5] f32 input
(V_GN, V_BGN, V_GD, V_BGD, V_BZ) = range(5)

# column layout of wpack [128, 128*9] bf16
(K_WSL, K_W1N, K_W1D, K_W2N, K_W2DF, K_M0T, K_M1T, K_M2T, K_U) = range(9)

BN_EPS = 1e-5
MSG_DRSWI = False   # SW-interleaved DoubleRow weights (contiguous LDW)


class Cfg:
    def __init__(self, N, E, C, sched):
        self.N = N
        self.E = E
        self.C = C
        self.F = 128
        assert N % C == 0
        self.npc = N // C
        self.W = (self.npc + 127) // 128
        assert self.W % 2 == 0
        self.WB = self.W // 2
        self.npad = self.W * 128
        # sched: tuple of ov counts per (w, ty), len W*2
        self.ov = np.asarray(sched, np.int64).reshape(self.W, 2)
        self.sched = tuple(int(v) for v in np.asarray(sched).ravel())
        # tiles per (w, ty) group and column offsets in the stream
        self.gtiles = KI + self.ov                  # [W, 2]
        # stream layout: per wb block: (w0,ty0),(w0,ty1),(w1,ty0),(w1,ty1)
        # each group: KI identity tiles then ov one-hot tiles
        gorder = []
        for wb in range(self.WB):
            for i in range(2):
                for ty in range(2):
                    gorder.append((2 * wb + i, ty))
        self.gorder = gorder
        col = 0
        ovt = 0
        self.gcol = np.zeros((self.W, 2), np.int64)    # tile col base
        self.govt = np.zeros((self.W, 2), np.int64)    # ov tile index base
        self.wb_base = []
        self.wb_cols = []
        for wb in range(self.WB):
            self.wb_base.append(col)
            for i in range(2):
                for ty in range(2):
                    w = 2 * wb + i
                    self.gcol[w, ty] = col
                    self.govt[w, ty] = ovt
                    col += int(self.gtiles[w, ty])
                    ovt += int(self.ov[w, ty])
            self.wb_cols.append(col - self.wb_base[-1])
        self.T = col                                   # total tiles/core
        self.OVT = ovt                                 # total ov tiles/core
        self.max_wb_tiles = max(self.wb_cols)
        self.max_wb_ovt = max(
            int(self.ov[2 * wb:2 * wb + 2].sum()) for wb in range(self.WB))


def build_schedule(dst, et, N, C):
    """ov[w][ty] = max over cores of ceil(overflow/128), with self-edges."""
    npc = N // C
    W = (npc + 127) // 128
    self_dst = np.arange(N, dtype=np.int64)
    dst_a = np.concatenate([self_dst, self_dst, dst])
    et_a = np.concatenate([np.zeros(N, np.int64), np.ones(N, np.int64), et])
    core = dst_a // npc
    ldst = dst_a - core * npc
    wdw = ldst >> 7
    j = ldst & 127
    key = ((core * W + wdw) * 2 + et_a) * 128 + j
    cnt = np.bincount(key, minlength=C * W * 2 * 128).reshape(C, W, 2, 128)
    excess = np.maximum(cnt - KI, 0).sum(axis=-1)       # [C, W, 2]
    ov = np.ceil(excess.max(axis=0) / 128).astype(np.int64)  # [W, 2]
    return tuple(int(v) for v in ov.ravel())


def build(cfg: Cfg):
    nc = bacc.Bacc("TRN2", target_bir_lowering=False, debug=False,
                   num_devices=cfg.C)
    W, WB, npad = cfg.W, cfg.WB, cfg.npad

    stream = nc.dram_tensor("stream", [128, cfg.T * 128], FP8,
                            kind="ExternalInput")
    xT = nc.dram_tensor("xT", [128, npad], BF16, kind="ExternalInput")
    sel = nc.dram_tensor("sel", [128, max(cfg.OVT, 1)], F32,
                         kind="ExternalInput")
    ipair = nc.dram_tensor("ipair", [128, 256], FP8, kind="ExternalInput")
    wpack = nc.dram_tensor("wpack", [128, 128 * 9], BF16,
                           kind="ExternalInput")
    rows = nc.dram_tensor("rows", [1, 512], BF16, kind="ExternalInput")
    vecs = nc.dram_tensor("vecs", [128, 5], F32, kind="ExternalInput")
    iota_in = nc.dram_tensor("iota128", [128, 128], F32,
                             kind="ExternalInput")
    out = nc.dram_tensor("out", [npad, 128], F32, kind="ExternalOutput")

    with tile.TileContext(nc) as tc:
        with (
            tc.tile_pool(name="res", bufs=1) as res,
            tc.tile_pool(name="msgp", bufs=6) as msgp,
            tc.tile_pool(name="sp", bufs=3) as sp,
            tc.tile_pool(name="hxp", bufs=4) as hxp,
            tc.tile_pool(name="sqp", bufs=3) as sqp,
            tc.tile_pool(name="smallp", bufs=8) as smallp,
            tc.tile_pool(name="dram", bufs=1, space="DRAM") as dram,
            tc.tile_pool(name="hbnp", bufs=4) as hbnp,
            tc.tile_pool(name="ep", bufs=3) as ep,
            tc.tile_pool(name="up", bufs=3) as up,
            tc.tile_pool(name="outp", bufs=3) as outp,
        ):
            # ---------- resident loads (first stream block leads) ----------
            msg0 = msgp.tile([128, cfg.max_wb_tiles * 128], FP8, tag="msg")
            # first block in four pieces so aggregation starts after the
            # first quarter lands instead of the full 2.7us transfer
            _b0 = cfg.wb_cols[0]
            _cuts = [0, _b0 // 4, _b0 // 2, 3 * _b0 // 4, _b0]
            for _a, _b in zip(_cuts[:-1], _cuts[1:]):
                nc.sync.dma_start(
                    msg0[:, _a * 128:_b * 128],
                    stream.ap()[:, _a * 128:_b * 128])
            ipair_sb = res.tile([128, 256], FP8)
            nc.sync.dma_start(ipair_sb[:], ipair.ap())
            # dummy warm-up collective: pays the ncfw/entry cost while
            # phase A runs, so the real stats allreduce is fast
            dummy_in = dram.tile([128, 1], F32)
            dummy_out = dram.tile([128, 1], F32)
            dz = smallp.tile([128, 1], F32, tag="dz")
            nc.vector.memset(dz[:], 0.0)
            nc.sync.dma_start(dummy_in[:], dz[:])
            nc.gpsimd.collective_compute(
                "AllReduce", OP.add,
                replica_groups=[list(range(cfg.C))],
                ins=[dummy_in[:].opt()], outs=[dummy_out[:].opt()],
            )
            wp = res.tile([128, 128 * 9], BF16)
            nc.sync.dma_start(wp[:], wpack.ap())
            sel_sb = res.tile([128, max(cfg.OVT, 1)], F32)
            nc.gpsimd.dma_start(sel_sb[:], sel.ap())
            rows_sb = res.tile([1, 512], BF16)
            nc.sync.dma_start(rows_sb[:], rows.ap())
            vec = res.tile([128, 5], F32)
            nc.sync.dma_start(vec[:], vecs.ap())
            iota_sb = res.tile([128, 128], F32)
            nc.gpsimd.dma_start(iota_sb[:], iota_in.ap())
            xT_sb = res.tile([128, npad], BF16)
            # xT is only read in phase C: load it inside the allreduce
            # shadow so it does not steal stream bandwidth in phase A

            h1n_sb = res.tile([128, npad], BF16)
            h1d_sb = res.tile([128, npad], BF16)
            ones_sb = res.tile([1, 128], BF16)
            nc.vector.memset(ones_sb[:], 1.0)
            stat_s = res.tile([128, 2 * WB], F32)
            stat_q = res.tile([128, 2 * WB], F32)
            junk = res.tile([128, 8], F32)
            cvec = res.tile([128, 2], F32)
            w2n_s = res.tile([128, 128], BF16)
            w2df_s = res.tile([128, 128], BF16)
            m1s = res.tile([128, 128], BF16)
            m2s = res.tile([128, 128], BF16)

            def wslice(k):
                return wp[:, k * 128:(k + 1) * 128]

            def vcol(k):
                return vec[:, k:k + 1]

            cc_in = dram.tile([128, 4], F32)
            cc_out = dram.tile([128, 4], F32)
            sums = res.tile([128, 4], F32)
            psums_s = res.tile([128, 2], F32)
            psums_q = res.tile([128, 2], F32)

            # ---------- phase A ----------
            with (
                tc.tile_pool(name="agg_ps", bufs=4, space="PSUM") as agg_psp,
                tc.tile_pool(name="h1_ps", bufs=3, space="PSUM") as h1_psp,
            ):
              def emit_h1(agg, wb):
                w0 = 2 * wb
                hx = hxp.tile([128, 512], BF16, tag="hx")
                nc.scalar.activation(hx[:, :], agg[:, :], ACT.Identity)
                h1 = h1_psp.tile([128, 512], F32, tag="h1")
                nc.tensor.matmul(h1[:, 0:256], lhsT=wslice(K_W1N),
                                 rhs=hx[:, 0:256], start=True, stop=False)
                nc.tensor.matmul(h1[:, 256:512], lhsT=wslice(K_W1D),
                                 rhs=hx[:, 256:512], start=False, stop=True)
                nsl = slice(w0 * 128, (w0 + 2) * 128)
                nc.scalar.activation(
                    h1n_sb[:, nsl], h1[:, 0:256], ACT.Identity,
                    accum_out=stat_s[:, 2 * wb:2 * wb + 1])
                nc.scalar.activation(
                    h1d_sb[:, nsl], h1[:, 256:512], ACT.Identity,
                    accum_out=stat_s[:, 2 * wb + 1:2 * wb + 2])
                sq = sqp.tile([128, 512], BF16, tag="sq")
                nc.vector.scalar_tensor_tensor(
                    out=sq[:, 0:256], in0=h1n_sb[:, nsl], scalar=1.0,
                    in1=h1n_sb[:, nsl], op0=OP.mult, op1=OP.mult,
                    accum_out=stat_q[:, 2 * wb:2 * wb + 1])
                nc.vector.scalar_tensor_tensor(
                    out=sq[:, 256:512], in0=h1d_sb[:, nsl], scalar=1.0,
                    in1=h1d_sb[:, nsl], op0=OP.mult, op1=OP.mult,
                    accum_out=stat_q[:, 2 * wb + 1:2 * wb + 2])

              pending = None
              for wb in range(WB):
                w0 = 2 * wb
                base = cfg.wb_base[wb]
                blk = cfg.wb_cols[wb]
                if wb == 0:
                    msg = msg0
                else:
                    msg = msgp.tile([128, cfg.max_wb_tiles * 128], FP8,
                                    tag="msg")
                    nc.sync.dma_start(
                        msg[:, 0:blk * 128],
                        stream.ap()[:, base * 128:(base + blk) * 128])
                # one-hot S for this wb's overflow tiles
                ovt0 = int(cfg.govt[w0, 0])
                novt = int(cfg.ov[w0:w0 + 2].sum())
                S = sp.tile([128, max(cfg.max_wb_ovt, 1) * 128], FP8,
                            tag="S")
                if novt:
                    nc.vector.tensor_tensor(
                        out=S[:, 0:novt * 128].rearrange(
                            "p (t j) -> p t j", j=128),
                        in0=iota_sb[:, :].rearrange("p (x j) -> p x j", x=1)
                            .to_broadcast([128, novt, 128]),
                        in1=sel_sb[:, ovt0:ovt0 + novt]
                            .to_broadcast([128, novt, 128]),
                        op=OP.is_equal,
                    )
                agg = agg_psp.tile([128, 512], F32, tag="agg")
                first = True
                ngrp = 0
                for i in range(2):
                    for ty in range(2):
                        w = w0 + i
                        dst_sl = slice((2 * ty + i) * 128,
                                       (2 * ty + i + 1) * 128)
                        mbase = (int(cfg.gcol[w, ty]) - base) * 128
                        nov = int(cfg.ov[w, ty])
                        last_grp = (ngrp == 3)
                        ngrp += 1
                        # 4 identity DR pairs
                        for t in range(0, KI, 2):
                            a = mbase + t * 128
                            lhs = (msg[:, a:a + 256] if MSG_DRSWI else
                                   msg[:, a:a + 256].rearrange(
                                       "p (t j) -> p t j", t=2))
                            nc.tensor.matmul(
                                agg[:, dst_sl], lhsT=lhs,
                                rhs=ipair_sb[:, :].rearrange(
                                    "p (t j) -> p t j", t=2),
                                perf_mode=(PM.DoubleRowSwInterleave
                                           if MSG_DRSWI else PM.DoubleRow),
                                start=first, stop=False)
                            first = False
                        # overflow tiles: DR pairs then maybe a single
                        sof = (int(cfg.govt[w, ty]) - ovt0) * 128
                        for t in range(0, nov - 1, 2):
                            a = mbase + (KI + t) * 128
                            s = sof + t * 128
                            lhs = (msg[:, a:a + 256] if MSG_DRSWI else
                                   msg[:, a:a + 256].rearrange(
                                       "p (t j) -> p t j", t=2))
                            nc.tensor.matmul(
                                agg[:, dst_sl], lhsT=lhs,
                                rhs=S[:, s:s + 256].rearrange(
                                    "p (t j) -> p t j", t=2),
                                perf_mode=(PM.DoubleRowSwInterleave
                                           if MSG_DRSWI else PM.DoubleRow),
                                start=False,
                                stop=(last_grp and t == nov - 2))
                        if nov % 2:
                            a = mbase + (KI + nov - 1) * 128
                            s = sof + (nov - 1) * 128
                            nc.tensor.matmul(
                                agg[:, dst_sl], lhsT=msg[:, a:a + 128],
                                rhs=S[:, s:s + 128],
                                start=False, stop=last_grp)
                if pending is not None:
                    emit_h1(*pending)
                    if pending[1] == WB - 2:
                        # partial stats over chunks [0, WB-1): runs while
                        # the last window-pair is still being aggregated
                        for br in range(2):
                            nc.vector.reduce_sum(
                                out=psums_s[:, br:br + 1],
                                in_=stat_s[:, 0:2 * (WB - 1)].rearrange(
                                    "p (w k) -> p w k", k=2)[:, :, br],
                                axis=AX.X)
                            nc.vector.reduce_sum(
                                out=psums_q[:, br:br + 1],
                                in_=stat_q[:, 0:2 * (WB - 1)].rearrange(
                                    "p (w k) -> p w k", k=2)[:, :, br],
                                axis=AX.X)
                pending = (agg, wb)
              emit_h1(*pending)

            # ---------- stats: single allreduce ----------
            last = 2 * (WB - 1)
            for br in range(2):
                nc.vector.tensor_tensor(
                    sums[:, 2 * br:2 * br + 1], psums_s[:, br:br + 1],
                    stat_s[:, last + br:last + br + 1], op=OP.add)
                nc.vector.tensor_tensor(
                    sums[:, 2 * br + 1:2 * br + 2], psums_q[:, br:br + 1],
                    stat_q[:, last + br:last + br + 1], op=OP.add)
            nc.sync.dma_start(cc_in[:], sums[:])
            nc.sync.dma_start(xT_sb[:], xT.ap())
            nc.gpsimd.collective_compute(
                "AllReduce", OP.add,
                replica_groups=[list(range(cfg.C))],
                ins=[cc_in[:].opt()], outs=[cc_out[:].opt()],
            )
            gsums = smallp.tile([128, 4], F32, tag="gsums")
            nc.sync.dma_start(gsums[:], cc_out[:])

            inv_n = 1.0 / cfg.N
            for br, (g_col, bg_col) in enumerate([(V_GN, V_BGN),
                                                  (V_GD, V_BGD)]):
                mean = smallp.tile([128, 1], F32, tag="mean")
                nc.vector.tensor_scalar(
                    out=mean[:], in0=gsums[:, 2 * br:2 * br + 1],
                    scalar1=inv_n, scalar2=None, op0=OP.mult)
                msq = smallp.tile([128, 1], F32, tag="msq")
                nc.vector.tensor_tensor(msq[:], mean[:], mean[:],
                                        op=OP.mult)
                var = smallp.tile([128, 1], F32, tag="var")
                nc.vector.tensor_scalar(
                    out=var[:], in0=gsums[:, 2 * br + 1:2 * br + 2],
                    scalar1=inv_n, scalar2=msq[:], op0=OP.mult,
                    op1=OP.subtract)
                nc.vector.tensor_scalar(out=var[:], in0=var[:],
                                        scalar1=BN_EPS, scalar2=None,
                                        op0=OP.add)
                # rstd = exp(-0.5*ln(var)), std = exp(+0.5*ln(var)):
                # keeps ACT on one table set (no Sqrt).
                lnv = smallp.tile([128, 1], F32, tag="lnv")
                nc.scalar.activation(lnv[:], var[:], ACT.Ln)
                rstd = smallp.tile([128, 1], F32, tag="rstd")
                nc.scalar.activation(rstd[:], lnv[:], ACT.Exp, scale=-0.5)
                std = smallp.tile([128, 1], F32, tag="std")
                nc.scalar.activation(std[:], lnv[:], ACT.Exp, scale=0.5)
                sc = smallp.tile([128, 1], F32, tag="sc")
                nc.vector.tensor_tensor(sc[:], vcol(g_col), rstd[:],
                                        op=OP.mult)
                # c = (beta/gamma)*std - mean
                nc.vector.tensor_scalar(
                    out=cvec[:, br:br + 1], in0=std[:],
                    scalar1=vcol(bg_col), scalar2=mean[:],
                    op0=OP.mult, op1=OP.subtract)
                wsl2 = wslice(K_W2N) if br == 0 else wslice(K_W2DF)
                wdst = w2n_s if br == 0 else w2df_s
                nc.scalar.activation(wdst[:], wsl2, ACT.Identity,
                                     scale=sc[:])
                msl = wslice(K_M1T) if br == 0 else wslice(K_M2T)
                mdst = m1s if br == 0 else m2s
                nc.vector.tensor_scalar(out=mdst[:], in0=msl,
                                        scalar1=sc[:], scalar2=None,
                                        op0=OP.mult)

            # ---------- phase C (software-pipelined) ----------
            with (
                tc.tile_pool(name="z_ps", bufs=2, space="PSUM") as z_psp,
                tc.tile_pool(name="nm_ps", bufs=3, space="PSUM") as nm_psp,
            ):
              NSB = (WB + 1) // 2
              hbn_t = [None] * NSB
              z_t = [None] * NSB
              e_t = [None] * NSB

              def nwb(sb):
                  return 2 if 2 * sb + 1 < WB else 1

              def emit_hbn(sb):
                  width = 256 * nwb(sb)
                  nsl = slice(4 * sb * 128, 4 * sb * 128 + width)
                  hbn = hbnp.tile([128, 1024], BF16, tag="hbn")
                  # relu(h1 + c); layout [n 4win | d 4win]
                  nc.scalar.activation(hbn[:, 0:width], h1n_sb[:, nsl],
                                       ACT.Relu, bias=cvec[:, 0:1])
                  nc.scalar.activation(hbn[:, 512:512 + width],
                                       h1d_sb[:, nsl],
                                       ACT.Relu, bias=cvec[:, 1:2])
                  hbn_t[sb] = hbn

              def emit_z_e(sb):
                  width = 256 * nwb(sb)
                  nsl = slice(4 * sb * 128, 4 * sb * 128 + width)
                  hbn = hbn_t[sb]
                  z = z_psp.tile([128, 512], F32, tag="z")
                  nc.tensor.matmul(z[:, 0:width], lhsT=wslice(K_M0T),
                                   rhs=xT_sb[:, nsl], start=True,
                                   stop=False)
                  nc.tensor.matmul(z[:, 0:width], lhsT=m1s[:],
                                   rhs=hbn[:, 0:width],
                                   start=False, stop=False)
                  nc.tensor.matmul(z[:, 0:width], lhsT=m2s[:],
                                   rhs=hbn[:, 512:512 + width],
                                   start=False, stop=True)
                  z_t[sb] = z
                  e = ep.tile([128, 512], BF16, tag="e")
                  nc.scalar.activation(e[:, 0:width], z[:, 0:width],
                                       ACT.Exp, bias=vcol(V_BZ))
                  e_t[sb] = e

              emit_hbn(0)
              emit_z_e(0)
              for sb in range(NSB):
               if sb + 1 < NSB:
                   emit_hbn(sb + 1)
                   emit_z_e(sb + 1)
               hbn, e = hbn_t[sb], e_t[sb]
               hbn_t[sb] = e_t[sb] = z_t[sb] = None
               for q in range(nwb(sb)):
                wb = 2 * sb + q
                w0 = 2 * wb
                # nm psum: per window [ct | At | xd | pad]
                nm = nm_psp.tile([128, 1024], F32, tag="nm")
                for i in range(2):
                    b = i * 512
                    k = 2 * q + i
                    isl = slice((w0 + i) * 128, (w0 + i + 1) * 128)
                    nc.tensor.matmul(nm[:, b:b + 128],
                                     lhsT=e[:, k * 128:(k + 1) * 128],
                                     rhs=wslice(K_U), start=True,
                                     stop=False)
                    nc.tensor.matmul(nm[:, b + 128:b + 256],
                                     lhsT=xT_sb[:, isl],
                                     rhs=wslice(K_WSL),
                                     start=False, stop=False)
                    nc.tensor.matmul(nm[:, b + 128:b + 256],
                                     lhsT=hbn[:, k * 128:(k + 1) * 128],
                                     rhs=w2n_s[:], start=False, stop=False)
                    nc.tensor.matmul(nm[:, b + 256:b + 384],
                                     lhsT=hbn[:, 512 + k * 128:
                                              512 + (k + 1) * 128],
                                     rhs=w2df_s[:], start=False,
                                     stop=False)
                # merged bias rank-1 over both windows' [At|xd] regions
                nc.tensor.matmul(
                    nm[:, :].rearrange("p (i q) -> p i q", i=2)[:, :,
                                                               128:384],
                    lhsT=ones_sb[:],
                    rhs=rows_sb[:, :].rearrange(
                        "r (i q) -> r i q", i=2)[:, :, 0:256],
                    start=False, stop=True, skip_group_check=True)
                nmv = nm[:, :].rearrange("p (i q) -> p i q", q=512)
                r = smallp.tile([128, 2], F32, tag="r")
                nc.vector.reciprocal(
                    r[:, :].rearrange("p (i u) -> p i u", u=1),
                    nmv[:, :, 127:128])
                t1 = up.tile([128, 256], BF16, tag="t1")
                for i in range(2):
                    nc.vector.tensor_scalar(
                        out=t1[:, i * 128:(i + 1) * 128],
                        in0=nm[:, i * 512:i * 512 + 128],
                        scalar1=r[:, i:i + 1], scalar2=None,
                        op0=OP.mult)
                u = up.tile([128, 256], BF16, tag="u")
                nc.vector.tensor_tensor(
                    out=u[:, :].rearrange("p (i q) -> p i q", i=2),
                    in0=t1[:, :].rearrange("p (i q) -> p i q", i=2),
                    in1=nmv[:, :, 256:384], op=OP.mult)
                o = outp.tile([128, 256], F32, tag="o")
                nc.vector.tensor_tensor(
                    out=o[:, :].rearrange("p (i q) -> p i q", i=2),
                    in0=u[:, :].rearrange("p (i q) -> p i q", i=2),
                    in1=nmv[:, :, 128:256], op=OP.add)
                nc.sync.dma_start(
                    out.ap()[w0 * 128:(w0 + 2) * 128, :]
                       .rearrange("(i p) f -> p i f", i=2),
                    o[:, :].rearrange("p (i f) -> p i f", i=2))

    nc.compile()
    return nc


def pack_edges(cfg: Cfg, src, dst, et):
    """Slot assignment incl. self-edges. Returns off [C,128,T] int32 and
    sel [C,128,OVT] bf16-ready float. Raises if schedule capacity exceeded.
    """
    C, W, npc = cfg.C, cfg.W, cfg.npc
    N = cfg.N
    self_idx = np.arange(N, dtype=np.int64)
    src_a = np.concatenate([self_idx, self_idx, src])
    dst_a = np.concatenate([self_idx, self_idx, dst])
    et_a = np.concatenate([np.zeros(N, np.int64), np.ones(N, np.int64), et])
    E = src_a.shape[0]
    core = dst_a // npc
    ldst = dst_a - core * npc
    wdw = ldst >> 7
    j = ldst & 127

    gkey = ((core * W + wdw) * 2 + et_a)
    fkey = gkey * 128 + j
    order = np.argsort(fkey, kind="stable")
    fs = fkey[order]
    gs = gkey[order]
    js = j[order]
    srcs = src_a[order]

    fcounts = np.bincount(fs, minlength=C * W * 2 * 128)
    fstarts = np.concatenate([[0], np.cumsum(fcounts)[:-1]])
    rank = np.arange(E, dtype=np.int64) - fstarts[fs]

    id_mask = rank < KI
    ov_mask = ~id_mask
    cum = np.cumsum(ov_mask)
    gcounts = np.bincount(gs, minlength=C * W * 2)
    gstarts = np.concatenate([[0], np.cumsum(gcounts)[:-1]])
    cum_at_start = np.where(gstarts > 0, cum[gstarts - 1], 0)
    ovr = cum - 1 - cum_at_start[gs]

    cores_s = (gs // (2 * W)).astype(np.int64)
    w_s = (gs // 2) % W
    ty_s = gs % 2

    # capacity check against the baked schedule
    ov_need = np.zeros((W, 2), np.int64)
    if ov_mask.any():
        np.maximum.at(ov_need, (w_s[ov_mask], ty_s[ov_mask]),
                      (ovr[ov_mask] >> 7) + 1)
    if (ov_need > cfg.ov).any():
        raise RuntimeError("overflow capacity exceeded vs schedule")

    tile_idx = np.where(id_mask, rank, KI + (ovr >> 7))
    part = np.where(id_mask, js, ovr & 127)
    col = cfg.gcol[w_s, ty_s] + tile_idx

    ZROW = cfg.N
    off = np.full((C, 128, cfg.T), ZROW, np.int32)
    off[cores_s, part, col] = srcs

    sel = np.full((C, 128, max(cfg.OVT, 1)), -1.0, np.float32)
    ov_idx = np.nonzero(ov_mask)[0]
    scol = cfg.govt[w_s[ov_idx], ty_s[ov_idx]] + (ovr[ov_idx] >> 7)
    sel[cores_s[ov_idx], ovr[ov_idx] & 127, scol] = js[ov_idx].astype(
        np.float32)
    return off, sel


def drswi_maps(cfg: Cfg):
    """Column permutation mapping plain [T*128] cols to the DRSWI stream.
    For each DR pair (tiles a,a+1): out col a*128 + 2k   = (a,   127-k)
                                    out col a*128 + 2k+1 = (a+1, 127-k).
    Single tiles keep plain layout."""
    T = cfg.T
    src_col = np.empty(T * 128, np.int64)
    for w in range(cfg.W):
        for ty in range(2):
            base = int(cfg.gcol[w, ty])
            nov = int(cfg.ov[w, ty])
            pairs = [(base + t, base + t + 1) for t in range(0, KI, 2)]
            pairs += [(base + KI + t, base + KI + t + 1)
                      for t in range(0, nov - 1, 2)]
            singles = ([base + KI + nov - 1] if nov % 2 else [])
            for (a, b) in pairs:
                k = np.arange(128)
                src_col[a * 128 + 2 * k] = a * 128 + (127 - k)
                src_col[a * 128 + 2 * k + 1] = b * 128 + (127 - k)
            for s in singles:
                src_col[s * 128:(s + 1) * 128] = np.arange(
                    s * 128, (s + 1) * 128)
    return src_col


def prep_inputs(cfg: Cfg, x, edge_index, edge_type, w_sl, b_sl,
                w1_n, b1_n, gamma_n, beta_n, w2_n, b2_n,
                w1_d, b1_d, gamma_d, beta_d, w2_d, b2_d,
                w_gat, b_gat):
    C, npc, npad = cfg.C, cfg.npc, cfg.npad
    x = np.asarray(x, np.float32)
    src = np.asarray(edge_index[0]).astype(np.int64)
    dst = np.asarray(edge_index[1]).astype(np.int64)
    et = np.asarray(edge_type).astype(np.int64)

    off, sel = pack_edges(cfg, src, dst, et)

    xf8 = np.vstack([x, np.zeros((1, 128), np.float32)]).astype(F8)
    perm = drswi_maps(cfg) if MSG_DRSWI else None
    streams = []
    for c in range(C):
        m = np.ascontiguousarray(xf8[off[c]].reshape(128, -1))
        if perm is not None:
            m = np.ascontiguousarray(m[:, perm])
        streams.append(m)

    xTs = []
    for c in range(C):
        xp = np.zeros((npad, 128), np.float32)
        xp[:npc] = x[c * npc:(c + 1) * npc]
        xTs.append(np.ascontiguousarray(xp.T).astype(BF))

    def bt(a):
        return np.ascontiguousarray(np.asarray(a, np.float64)).astype(BF)

    w_sl64 = np.asarray(w_sl, np.float64)
    w2n64 = np.asarray(w2_n, np.float64)
    w2d64 = np.asarray(w2_d, np.float64)
    wg = np.asarray(w_gat, np.float64)
    wg0, wg1, wg2 = wg[:, 0:128], wg[:, 128:256], wg[:, 256:384]

    wcols = [
        bt(w_sl64.T), bt(np.asarray(w1_n).T), bt(np.asarray(w1_d).T),
        bt(w2n64.T), bt(w2d64[::-1, :].T),
        bt((wg0 @ w_sl64).T), bt((wg1 @ w2n64).T), bt((wg2 @ w2d64).T),
        bt(np.triu(np.ones((128, 128), np.float32))),
    ]
    wpack = np.concatenate(wcols, axis=1)

    bias_at = (np.asarray(b_sl, np.float64)
               + np.asarray(b2_n, np.float64))
    bias_xd = np.asarray(b2_d, np.float64)[::-1]
    rows = np.concatenate([bias_at, bias_xd, bias_at, bias_xd]
                          )[None, :].astype(BF)

    bz = (np.asarray(b_gat, np.float64) + wg0 @ np.asarray(b_sl, np.float64)
          + wg1 @ np.asarray(b2_n, np.float64)
          + wg2 @ np.asarray(b2_d, np.float64))
    g_n = np.asarray(gamma_n, np.float64)
    g_d = np.asarray(gamma_d, np.float64)
    vecs = np.stack([
        g_n, np.asarray(beta_n, np.float64) / g_n,
        g_d, np.asarray(beta_d, np.float64) / g_d,
        bz,
    ], axis=1).astype(np.float32)

    in_maps = []
    for c in range(C):
        m = {
            "stream": streams[c],
            "xT": xTs[c],
            "sel": np.ascontiguousarray(sel[c]),
            "wpack": wpack,
            "rows": rows,
            "vecs": vecs,
            "iota128": np.broadcast_to(
                np.arange(128, dtype=np.float32)[None, :],
                (128, 128)).copy(),
            "ipair": np.concatenate(
                [np.eye(128, dtype=np.float32)] * 2, axis=1).astype(F8),
        }
        in_maps.append(m)
    return in_maps


_BUILD_CACHE = {}


def make_cfg(inputs):
    dst = np.asarray(inputs["edge_index"][1]).astype(np.int64)
    et = np.asarray(inputs["edge_type"]).astype(np.int64)
    sched = build_schedule(dst, et, N_GLOBAL, CORES)
    return Cfg(N_GLOBAL, E_GLOBAL, CORES, sched)


def run(cfg: Cfg, inputs: dict, **run_kwargs):
    in_maps = prep_inputs(cfg, **inputs)
    key = (cfg.N, cfg.E, cfg.C, MSG_DRSWI, cfg.sched)
    if key not in _BUILD_CACHE:
        _BUILD_CACHE[key] = build(cfg)
    nc = _BUILD_CACHE[key]
    res = run_bass_kernel_spmd(nc, in_maps, core_ids=list(range(cfg.C)),
                               **run_kwargs)
    outs = [res.results[c]["out"][:cfg.npc] for c in range(cfg.C)]
    return np.concatenate(outs, axis=0).astype(np.float32), res


def kernel(**inputs):
    out, _ = run(make_cfg(inputs), inputs)
    return out


# revision 9
# speedup vs baseline: 1.0163x; 1.0163x over previous
"""Trainium2 Bass kernel for the 2-relation GIN GNN layer (final).

Design highlights (each validated against neuron-profile traces):
  - edges packed per destination window as fp8 identity tiles (slot
    partition = dst) + variable one-hot overflow tiles baked from the
    graph (max over cores so the SPMD program is shared); self-edges
    folded into the same slot stream; scatter-sum via DoubleRow fp8
    matmuls against a resident identity pair / on-device one-hot S.
  - phase A software-pipelined: h1 matmuls run one window-pair behind
    the aggregation so the PE never waits on the ACT psum->sbuf copy.
  - BN batch stats via ACT copy-accumulate + DVE square-accumulate;
    ONE AllReduce at the end; a dummy warm-up collective at kernel
    start pays the ncfw entry cost, and rstd = exp(-0.5*ln(var)) keeps
    the scalar engine on a single activation table (no table reloads).
  - xT (feature-major x, needed only in phase C) is loaded inside the
    allreduce shadow: the stream DMA then owns all HBM bandwidth in
    phase A, and the collective gap's idle DMA does useful work.
  - phase C: gate logits via composed CPU-side weights; hbn/z/e
    emitted one super-block (2 window-pairs) ahead of the nm matmuls;
    one merged FD=512 rank-1 bias matmul per window-pair; cumsum via
    triangular-ones matmul; normalize/combine on DVE.
"""

import numpy as np
import ml_dtypes

import concourse.bass as bass
import concourse.mybir as mybir
import concourse.tile as tile
from concourse import bacc
from concourse.bass_utils import run_bass_kernel_spmd

F32 = mybir.dt.float32
BF16 = mybir.dt.bfloat16
FP8 = mybir.dt.float8e4
AX = mybir.AxisListType
OP = mybir.AluOpType
ACT = mybir.ActivationFunctionType
PM = mybir.MatmulPerfMode

BF = ml_dtypes.bfloat16
F8 = ml_dtypes.float8_e4m3

N_GLOBAL = 100000
E_GLOBAL = 1600000
CORES = 8
KI = 8

# column layout of the "vecs" [128, 5] f32 input
(V_GN, V_BGN, V_GD, V_BGD, V_BZ) = range(5)

# column layout of wpack [128, 128*9] bf16
(K_WSL, K_W1N, K_W1D, K_W2N, K_W2DF, K_M0T, K_M1T, K_M2T, K_U) = range(9)

BN_EPS = 1e-5
MSG_DRSWI = False   # SW-interleaved DoubleRow weights (contiguous LDW)


class Cfg:
    def __init__(self, N, E, C, sched):
        self.N = N
        self.E = E
        self.C = C
        self.F = 128
        assert N % C == 0
        self.npc = N // C
        self.W = (self.npc + 127) // 128
        assert self.W % 2 == 0
        self.WB = self.W // 2
        self.npad = self.W * 128
        # sched: tuple of ov counts per (w, ty), len W*2
        self.ov = np.asarray(sched, np.int64).reshape(self.W, 2)
        self.sched = tuple(int(v) for v in np.asarray(sched).ravel())
        # tiles per (w, ty) group and column offsets in the stream
        self.gtiles = KI + self.ov                  # [W, 2]
        # stream layout: per wb block: (w0,ty0),(w0,ty1),(w1,ty0),(w1,ty1)
        # each group: KI identity tiles then ov one-hot tiles
        gorder = []
        for wb in range(self.WB):
            for i in range(2):
                for ty in range(2):
                    gorder.append((2 * wb + i, ty))
        self.gorder = gorder
        col = 0
        ovt = 0
        self.gcol = np.zeros((self.W, 2), np.int64)    # tile col base
        self.govt = np.zeros((self.W, 2), np.int64)    # ov tile index base
        self.wb_base = []
        self.wb_cols = []
        for wb in range(self.WB):
            self.wb_base.append(col)
            for i in range(2):
                for ty in range(2):
                    w = 2 * wb + i
                    self.gcol[w, ty] = col
                    self.govt[w, ty] = ovt
                    col += int(self.gtiles[w, ty])
                    ovt += int(self.ov[w, ty])
            self.wb_cols.append(col - self.wb_base[-1])
        self.T = col                                   # total tiles/core
        self.OVT = ovt                                 # total ov tiles/core
        self.max_wb_tiles = max(self.wb_cols)
        self.max_wb_ovt = max(
            int(self.ov[2 * wb:2 * wb + 2].sum()) for wb in range(self.WB))


def build_schedule(dst, et, N, C):
    """ov[w][ty] = max over cores of ceil(overflow/128), with self-edges."""
    npc = N // C
    W = (npc + 127) // 128
    self_dst = np.arange(N, dtype=np.int64)
    dst_a = np.concatenate([self_dst, self_dst, dst])
    et_a = np.concatenate([np.zeros(N, np.int64), np.ones(N, np.int64), et])
    core = dst_a // npc
    ldst = dst_a - core * npc
    wdw = ldst >> 7
    j = ldst & 127
    key = ((core * W + wdw) * 2 + et_a) * 128 + j
    cnt = np.bincount(key, minlength=C * W * 2 * 128).reshape(C, W, 2, 128)
    excess = np.maximum(cnt - KI, 0).sum(axis=-1)       # [C, W, 2]
    ov = np.ceil(excess.max(axis=0) / 128).astype(np.int64)  # [W, 2]
    return tuple(int(v) for v in ov.ravel())


def build(cfg: Cfg):
    nc = bacc.Bacc("TRN2", target_bir_lowering=False, debug=False,
                   num_devices=cfg.C)
    W, WB, npad = cfg.W, cfg.WB, cfg.npad

    stream = nc.dram_tensor("stream", [128, cfg.T * 128], FP8,
                            kind="ExternalInput")
    xT = nc.dram_tensor("xT", [128, npad], BF16, kind="ExternalInput")
    sel = nc.dram_tensor("sel", [128, max(cfg.OVT, 1)], F32,
                         kind="ExternalInput")
    ipair = nc.dram_tensor("ipair", [128, 256], FP8, kind="ExternalInput")
    wpack = nc.dram_tensor("wpack", [128, 128 * 9], BF16,
                           kind="ExternalInput")
    rows = nc.dram_tensor("rows", [1, 512], BF16, kind="ExternalInput")
    vecs = nc.dram_tensor("vecs", [128, 5], F32, kind="ExternalInput")
    iota_in = nc.dram_tensor("iota128", [128, 128], F32,
                             kind="ExternalInput")
    out = nc.dram_tensor("out", [npad, 128], F32, kind="ExternalOutput")

    with tile.TileContext(nc) as tc:
        with (
            tc.tile_pool(name="res", bufs=1) as res,
            tc.tile_pool(name="msgp", bufs=6) as msgp,
            tc.tile_pool(name="sp", bufs=3) as sp,
            tc.tile_pool(name="hxp", bufs=4) as hxp,
            tc.tile_pool(name="sqp", bufs=3) as sqp,
            tc.tile_pool(name="smallp", bufs=8) as smallp,
            tc.tile_pool(name="dram", bufs=1, space="DRAM") as dram,
            tc.tile_pool(name="hbnp", bufs=4) as hbnp,
            tc.tile_pool(name="ep", bufs=3) as ep,
            tc.tile_pool(name="up", bufs=3) as up,
            tc.tile_pool(name="outp", bufs=3) as outp,
        ):
            # ---------- resident loads (first stream block leads) ----------
            msg0 = msgp.tile([128, cfg.max_wb_tiles * 128], FP8, tag="msg")
            # first block in four pieces so aggregation starts after the
            # first quarter lands instead of the full 2.7us transfer
            _b0 = cfg.wb_cols[0]
            _cuts = [0, _b0 // 4, _b0 // 2, 3 * _b0 // 4, _b0]
            for _a, _b in zip(_cuts[:-1], _cuts[1:]):
                nc.sync.dma_start(
                    msg0[:, _a * 128:_b * 128],
                    stream.ap()[:, _a * 128:_b * 128])
            ipair_sb = res.tile([128, 256], FP8)
            nc.sync.dma_start(ipair_sb[:], ipair.ap())
            # dummy warm-up collective: pays the ncfw/entry cost while
            # phase A runs, so the real stats allreduce is fast
            dummy_in = dram.tile([128, 1], F32)
            dummy_out = dram.tile([128, 1], F32)
            dz = smallp.tile([128, 1], F32, tag="dz")
            nc.vector.memset(dz[:], 0.0)
            nc.sync.dma_start(dummy_in[:], dz[:])
            nc.gpsimd.collective_compute(
                "AllReduce", OP.add,
                replica_groups=[list(range(cfg.C))],
                ins=[dummy_in[:].opt()], outs=[dummy_out[:].opt()],
            )
            wp = res.tile([128, 128 * 9], BF16)
            nc.sync.dma_start(wp[:], wpack.ap())
            sel_sb = res.tile([128, max(cfg.OVT, 1)], F32)
            nc.gpsimd.dma_start(sel_sb[:], sel.ap())
            rows_sb = res.tile([1, 512], BF16)
            nc.sync.dma_start(rows_sb[:], rows.ap())
            vec = res.tile([128, 5], F32)
            nc.sync.dma_start(vec[:], vecs.ap())
            iota_sb = res.tile([128, 128], F32)
            nc.gpsimd.dma_start(iota_sb[:], iota_in.ap())
            xT_sb = res.tile([128, npad], BF16)
            # xT is only read in phase C: load it inside the allreduce
            # shadow so it does not steal stream bandwidth in phase A

            h1n_sb = res.tile([128, npad], BF16)
            h1d_sb = res.tile([128, npad], BF16)
            ones_sb = res.tile([1, 128], BF16)
            nc.vector.memset(ones_sb[:], 1.0)
            stat_s = res.tile([128, 2 * WB], F32)
            stat_q = res.tile([128, 2 * WB], F32)
            junk = res.tile([128, 8], F32)
            cvec = res.tile([128, 2], F32)
            w2n_s = res.tile([128, 128], BF16)
            w2df_s = res.tile([128, 128], BF16)
            m1s = res.tile([128, 128], BF16)
            m2s = res.tile([128, 128], BF16)

            def wslice(k):
                return wp[:, k * 128:(k + 1) * 128]

            def vcol(k):
                return vec[:, k:k + 1]

            cc_in = dram.tile([128, 4], F32)
            cc_out = dram.tile([128, 4], F32)
            sums = res.tile([128, 4], F32)
            psums_s = res.tile([128, 2], F32)
            psums_q = res.tile([128, 2], F32)

            # ---------- phase A ----------
            with (
                tc.tile_pool(name="agg_ps", bufs=4, space="PSUM") as agg_psp,
                tc.tile_pool(name="h1_ps", bufs=3, space="PSUM") as h1_psp,
            ):
              def emit_h1(agg, wb):
                w0 = 2 * wb
                hx = hxp.tile([128, 512], BF16, tag="hx")
                nc.scalar.activation(hx[:, :], agg[:, :], ACT.Identity)
                h1 = h1_psp.tile([128, 512], F32, tag="h1")
                nc.tensor.matmul(h1[:, 0:256], lhsT=wslice(K_W1N),
                                 rhs=hx[:, 0:256], start=True, stop=False)
                nc.tensor.matmul(h1[:, 256:512], lhsT=wslice(K_W1D),
                                 rhs=hx[:, 256:512], start=False, stop=True)
                nsl = slice(w0 * 128, (w0 + 2) * 128)
                nc.scalar.activation(
                    h1n_sb[:, nsl], h1[:, 0:256], ACT.Identity,
                    accum_out=stat_s[:, 2 * wb:2 * wb + 1])
                nc.scalar.activation(
                    h1d_sb[:, nsl], h1[:, 256:512], ACT.Identity,
                    accum_out=stat_s[:, 2 * wb + 1:2 * wb + 2])
                sq = sqp.tile([128, 512], BF16, tag="sq")
                nc.vector.scalar_tensor_tensor(
                    out=sq[:, 0:256], in0=h1n_sb[:, nsl], scalar=1.0,
                    in1=h1n_sb[:, nsl], op0=OP.mult, op1=OP.mult,
                    accum_out=stat_q[:, 2 * wb:2 * wb + 1])
                nc.vector.scalar_tensor_tensor(
                    out=sq[:, 256:512], in0=h1d_sb[:, nsl], scalar=1.0,
                    in1=h1d_sb[:, nsl], op0=OP.mult, op1=OP.mult,
                    accum_out=stat_q[:, 2 * wb + 1:2 * wb + 2])

              pending = None
              for wb in range(WB):
                w0 = 2 * wb
                base = cfg.wb_base[wb]
                blk = cfg.wb_cols[wb]
                if wb == 0:
                    msg = msg0
                else:
                    msg = msgp.tile([128, cfg.max_wb_tiles * 128], FP8,
                                    tag="msg")
                    if wb <= 3:
                        # ramp blocks in halves: finer arrival granularity
                        h = blk // 2
                        nc.sync.dma_start(
                            msg[:, 0:h * 128],
                            stream.ap()[:, base * 128:(base + h) * 128])
                        nc.sync.dma_start(
                            msg[:, h * 128:blk * 128],
                            stream.ap()[:, (base + h) * 128:
                                        (base + blk) * 128])
                    else:
                        nc.sync.dma_start(
                            msg[:, 0:blk * 128],
                            stream.ap()[:, base * 128:(base + blk) * 128])
                # one-hot S for this wb's overflow tiles
                ovt0 = int(cfg.govt[w0, 0])
                novt = int(cfg.ov[w0:w0 + 2].sum())
                S = sp.tile([128, max(cfg.max_wb_ovt, 1) * 128], FP8,
                            tag="S")
                if novt:
                    nc.vector.tensor_tensor(
                        out=S[:, 0:novt * 128].rearrange(
                            "p (t j) -> p t j", j=128),
                        in0=iota_sb[:, :].rearrange("p (x j) -> p x j", x=1)
                            .to_broadcast([128, novt, 128]),
                        in1=sel_sb[:, ovt0:ovt0 + novt]
                            .to_broadcast([128, novt, 128]),
                        op=OP.is_equal,
                    )
                agg = agg_psp.tile([128, 512], F32, tag="agg")
                first = True
                ngrp = 0
                for i in range(2):
                    for ty in range(2):
                        w = w0 + i
                        dst_sl = slice((2 * ty + i) * 128,
                                       (2 * ty + i + 1) * 128)
                        mbase = (int(cfg.gcol[w, ty]) - base) * 128
                        nov = int(cfg.ov[w, ty])
                        last_grp = (ngrp == 3)
                        ngrp += 1
                        # 4 identity DR pairs
                        for t in range(0, KI, 2):
                            a = mbase + t * 128
                            lhs = (msg[:, a:a + 256] if MSG_DRSWI else
                                   msg[:, a:a + 256].rearrange(
                                       "p (t j) -> p t j", t=2))
                            nc.tensor.matmul(
                                agg[:, dst_sl], lhsT=lhs,
                                rhs=ipair_sb[:, :].rearrange(
                                    "p (t j) -> p t j", t=2),
                                perf_mode=(PM.DoubleRowSwInterleave
                                           if MSG_DRSWI else PM.DoubleRow),
                                start=first, stop=False)
                            first = False
                        # overflow tiles: DR pairs then maybe a single
                        sof = (int(cfg.govt[w, ty]) - ovt0) * 128
                        for t in range(0, nov - 1, 2):
                            a = mbase + (KI + t) * 128
                            s = sof + t * 128
                            lhs = (msg[:, a:a + 256] if MSG_DRSWI else
                                   msg[:, a:a + 256].rearrange(
                                       "p (t j) -> p t j", t=2))
                            nc.tensor.matmul(
                                agg[:, dst_sl], lhsT=lhs,
                                rhs=S[:, s:s + 256].rearrange(
                                    "p (t j) -> p t j", t=2),
                                perf_mode=(PM.DoubleRowSwInterleave
                                           if MSG_DRSWI else PM.DoubleRow),
                                start=False,
                                stop=(last_grp and t == nov - 2))
                        if nov % 2:
                            a = mbase + (KI + nov - 1) * 128
                            s = sof + (nov - 1) * 128
                            nc.tensor.matmul(
                                agg[:, dst_sl], lhsT=msg[:, a:a + 128],
                                rhs=S[:, s:s + 128],
                                start=False, stop=last_grp)
                if pending is not None:
                    emit_h1(*pending)
                    if pending[1] == WB - 2:
                        # partial stats over chunks [0, WB-1): runs while
                        # the last window-pair is still being aggregated
                        for br in range(2):
                            nc.vector.reduce_sum(
                                out=psums_s[:, br:br + 1],
                                in_=stat_s[:, 0:2 * (WB - 1)].rearrange(
                                    "p (w k) -> p w k", k=2)[:, :, br],
                                axis=AX.X)
                            nc.vector.reduce_sum(
                                out=psums_q[:, br:br + 1],
                                in_=stat_q[:, 0:2 * (WB - 1)].rearrange(
                                    "p (w k) -> p w k", k=2)[:, :, br],
                                axis=AX.X)
                pending = (agg, wb)
              emit_h1(*pending)

            # ---------- stats: single allreduce ----------
            last = 2 * (WB - 1)
            for br in range(2):
                nc.vector.tensor_tensor(
                    sums[:, 2 * br:2 * br + 1], psums_s[:, br:br + 1],
                    stat_s[:, last + br:last + br + 1], op=OP.add)
                nc.vector.tensor_tensor(
                    sums[:, 2 * br + 1:2 * br + 2], psums_q[:, br:br + 1],
                    stat_q[:, last + br:last + br + 1], op=OP.add)
            nc.sync.dma_start(cc_in[:], sums[:])
            nc.sync.dma_start(xT_sb[:], xT.ap())
            nc.gpsimd.collective_compute(
                "AllReduce", OP.add,
                replica_groups=[list(range(cfg.C))],
                ins=[cc_in[:].opt()], outs=[cc_out[:].opt()],
            )
            gsums = smallp.tile([128, 4], F32, tag="gsums")
            nc.sync.dma_start(gsums[:], cc_out[:])

            inv_n = 1.0 / cfg.N
            for br, (g_col, bg_col) in enumerate([(V_GN, V_BGN),
                                                  (V_GD, V_BGD)]):
                mean = smallp.tile([128, 1], F32, tag="mean")
                nc.vector.tensor_scalar(
                    out=mean[:], in0=gsums[:, 2 * br:2 * br + 1],
                    scalar1=inv_n, scalar2=None, op0=OP.mult)
                msq = smallp.tile([128, 1], F32, tag="msq")
                nc.vector.tensor_tensor(msq[:], mean[:], mean[:],
                                        op=OP.mult)
                var = smallp.tile([128, 1], F32, tag="var")
                nc.vector.tensor_scalar(
                    out=var[:], in0=gsums[:, 2 * br + 1:2 * br + 2],
                    scalar1=inv_n, scalar2=msq[:], op0=OP.mult,
                    op1=OP.subtract)
                nc.vector.tensor_scalar(out=var[:], in0=var[:],
                                        scalar1=BN_EPS, scalar2=None,
                                        op0=OP.add)
                # rstd = exp(-0.5*ln(var)), std = exp(+0.5*ln(var)):
                # keeps ACT on one table set (no Sqrt).
                lnv = smallp.tile([128, 1], F32, tag="lnv")
                nc.scalar.activation(lnv[:], var[:], ACT.Ln)
                rstd = smallp.tile([128, 1], F32, tag="rstd")
                nc.scalar.activation(rstd[:], lnv[:], ACT.Exp, scale=-0.5)
                std = smallp.tile([128, 1], F32, tag="std")
                nc.scalar.activation(std[:], lnv[:], ACT.Exp, scale=0.5)
                sc = smallp.tile([128, 1], F32, tag="sc")
                nc.vector.tensor_tensor(sc[:], vcol(g_col), rstd[:],
                                        op=OP.mult)
                # c = (beta/gamma)*std - mean
                nc.vector.tensor_scalar(
                    out=cvec[:, br:br + 1], in0=std[:],
                    scalar1=vcol(bg_col), scalar2=mean[:],
                    op0=OP.mult, op1=OP.subtract)
                wsl2 = wslice(K_W2N) if br == 0 else wslice(K_W2DF)
                wdst = w2n_s if br == 0 else w2df_s
                nc.scalar.activation(wdst[:], wsl2, ACT.Identity,
                                     scale=sc[:])
                msl = wslice(K_M1T) if br == 0 else wslice(K_M2T)
                mdst = m1s if br == 0 else m2s
                nc.vector.tensor_scalar(out=mdst[:], in0=msl,
                                        scalar1=sc[:], scalar2=None,
                                        op0=OP.mult)

            # ---------- phase C (software-pipelined) ----------
            with (
                tc.tile_pool(name="z_ps", bufs=2, space="PSUM") as z_psp,
                tc.tile_pool(name="nm_ps", bufs=3, space="PSUM") as nm_psp,
            ):
              NSB = (WB + 1) // 2
              hbn_t = [None] * NSB
              z_t = [None] * NSB
              e_t = [None] * NSB

              def nwb(sb):
                  return 2 if 2 * sb + 1 < WB else 1

              def emit_hbn(sb):
                  width = 256 * nwb(sb)
                  nsl = slice(4 * sb * 128, 4 * sb * 128 + width)
                  hbn = hbnp.tile([128, 1024], BF16, tag="hbn")
                  # relu(h1 + c); layout [n 4win | d 4win]
                  nc.scalar.activation(hbn[:, 0:width], h1n_sb[:, nsl],
                                       ACT.Relu, bias=cvec[:, 0:1])
                  nc.scalar.activation(hbn[:, 512:512 + width],
                                       h1d_sb[:, nsl],
                                       ACT.Relu, bias=cvec[:, 1:2])
                  hbn_t[sb] = hbn

              def emit_z_e(sb):
                  width = 256 * nwb(sb)
                  nsl = slice(4 * sb * 128, 4 * sb * 128 + width)
                  hbn = hbn_t[sb]
                  z = z_psp.tile([128, 512], F32, tag="z")
                  nc.tensor.matmul(z[:, 0:width], lhsT=wslice(K_M0T),
                                   rhs=xT_sb[:, nsl], start=True,
                                   stop=False)
                  nc.tensor.matmul(z[:, 0:width], lhsT=m1s[:],
                                   rhs=hbn[:, 0:width],
                                   start=False, stop=False)
                  nc.tensor.matmul(z[:, 0:width], lhsT=m2s[:],
                                   rhs=hbn[:, 512:512 + width],
                                   start=False, stop=True)
                  z_t[sb] = z
                  e = ep.tile([128, 512], BF16, tag="e")
                  nc.scalar.activation(e[:, 0:width], z[:, 0:width],
                                       ACT.Exp, bias=vcol(V_BZ))
                  e_t[sb] = e

              emit_hbn(0)
              emit_z_e(0)
              for sb in range(NSB):
               if sb + 1 < NSB:
                   emit_hbn(sb + 1)
                   emit_z_e(sb + 1)
               hbn, e = hbn_t[sb], e_t[sb]
               hbn_t[sb] = e_t[sb] = z_t[sb] = None
               for q in range(nwb(sb)):
                wb = 2 * sb + q
                w0 = 2 * wb
                # nm psum: per window [ct | At | xd | pad]
                nm = nm_psp.tile([128, 1024], F32, tag="nm")
                for i in range(2):
                    b = i * 512
                    k = 2 * q + i
                    isl = slice((w0 + i) * 128, (w0 + i + 1) * 128)
                    nc.tensor.matmul(nm[:, b:b + 128],
                                     lhsT=e[:, k * 128:(k + 1) * 128],
                                     rhs=wslice(K_U), start=True,
                                     stop=False)
                    nc.tensor.matmul(nm[:, b + 128:b + 256],
                                     lhsT=xT_sb[:, isl],
                                     rhs=wslice(K_WSL),
                                     start=False, stop=False)
                    nc.tensor.matmul(nm[:, b + 128:b + 256],
                                     lhsT=hbn[:, k * 128:(k + 1) * 128],
                                     rhs=w2n_s[:], start=False, stop=False)
                    nc.tensor.matmul(nm[:, b + 256:b + 384],
                                     lhsT=hbn[:, 512 + k * 128:
                                              512 + (k + 1) * 128],
                                     rhs=w2df_s[:], start=False,
                                     stop=False)
                # merged bias rank-1 over both windows' [At|xd] regions
                nc.tensor.matmul(
                    nm[:, :].rearrange("p (i q) -> p i q", i=2)[:, :,
                                                               128:384],
                    lhsT=ones_sb[:],
                    rhs=rows_sb[:, :].rearrange(
                        "r (i q) -> r i q", i=2)[:, :, 0:256],
                    start=False, stop=True, skip_group_check=True)
                nmv = nm[:, :].rearrange("p (i q) -> p i q", q=512)
                r = smallp.tile([128, 2], F32, tag="r")
                nc.vector.reciprocal(
                    r[:, :].rearrange("p (i u) -> p i u", u=1),
                    nmv[:, :, 127:128])
                t1 = up.tile([128, 256], BF16, tag="t1")
                for i in range(2):
                    nc.vector.tensor_scalar(
                        out=t1[:, i * 128:(i + 1) * 128],
                        in0=nm[:, i * 512:i * 512 + 128],
                        scalar1=r[:, i:i + 1], scalar2=None,
                        op0=OP.mult)
                u = up.tile([128, 256], BF16, tag="u")
                nc.vector.tensor_tensor(
                    out=u[:, :].rearrange("p (i q) -> p i q", i=2),
                    in0=t1[:, :].rearrange("p (i q) -> p i q", i=2),
                    in1=nmv[:, :, 256:384], op=OP.mult)
                o = outp.tile([128, 256], F32, tag="o")
                nc.vector.tensor_tensor(
                    out=o[:, :].rearrange("p (i q) -> p i q", i=2),
                    in0=u[:, :].rearrange("p (i q) -> p i q", i=2),
                    in1=nmv[:, :, 128:256], op=OP.add)
                nc.sync.dma_start(
                    out.ap()[w0 * 128:(w0 + 2) * 128, :]
                       .rearrange("(i p) f -> p i f", i=2),
                    o[:, :].rearrange("p (i f) -> p i f", i=2))

    nc.compile()
    return nc


def pack_edges(cfg: Cfg, src, dst, et):
    """Slot assignment incl. self-edges. Returns off [C,128,T] int32 and
    sel [C,128,OVT] bf16-ready float. Raises if schedule capacity exceeded.
    """
    C, W, npc = cfg.C, cfg.W, cfg.npc
    N = cfg.N
    self_idx = np.arange(N, dtype=np.int64)
    src_a = np.concatenate([self_idx, self_idx, src])
    dst_a = np.concatenate([self_idx, self_idx, dst])
    et_a = np.concatenate([np.zeros(N, np.int64), np.ones(N, np.int64), et])
    E = src_a.shape[0]
    core = dst_a // npc
    ldst = dst_a - core * npc
    wdw = ldst >> 7
    j = ldst & 127

    gkey = ((core * W + wdw) * 2 + et_a)
    fkey = gkey * 128 + j
    order = np.argsort(fkey, kind="stable")
    fs = fkey[order]
    gs = gkey[order]
    js = j[order]
    srcs = src_a[order]

    fcounts = np.bincount(fs, minlength=C * W * 2 * 128)
    fstarts = np.concatenate([[0], np.cumsum(fcounts)[:-1]])
    rank = np.arange(E, dtype=np.int64) - fstarts[fs]

    id_mask = rank < KI
    ov_mask = ~id_mask
    cum = np.cumsum(ov_mask)
    gcounts = np.bincount(gs, minlength=C * W * 2)
    gstarts = np.concatenate([[0], np.cumsum(gcounts)[:-1]])
    cum_at_start = np.where(gstarts > 0, cum[gstarts - 1], 0)
    ovr = cum - 1 - cum_at_start[gs]

    cores_s = (gs // (2 * W)).astype(np.int64)
    w_s = (gs // 2) % W
    ty_s = gs % 2

    # capacity check against the baked schedule
    ov_need = np.zeros((W, 2), np.int64)
    if ov_mask.any():
        np.maximum.at(ov_need, (w_s[ov_mask], ty_s[ov_mask]),
                      (ovr[ov_mask] >> 7) + 1)
    if (ov_need > cfg.ov).any():
        raise RuntimeError("overflow capacity exceeded vs schedule")

    tile_idx = np.where(id_mask, rank, KI + (ovr >> 7))
    part = np.where(id_mask, js, ovr & 127)
    col = cfg.gcol[w_s, ty_s] + tile_idx

    ZROW = cfg.N
    off = np.full((C, 128, cfg.T), ZROW, np.int32)
    off[cores_s, part, col] = srcs

    sel = np.full((C, 128, max(cfg.OVT, 1)), -1.0, np.float32)
    ov_idx = np.nonzero(ov_mask)[0]
    scol = cfg.govt[w_s[ov_idx], ty_s[ov_idx]] + (ovr[ov_idx] >> 7)
    sel[cores_s[ov_idx], ovr[ov_idx] & 127, scol] = js[ov_idx].astype(
        np.float32)
    return off, sel


def drswi_maps(cfg: Cfg):
    """Column permutation mapping plain [T*128] cols to the DRSWI stream.
    For each DR pair (tiles a,a+1): out col a*128 + 2k   = (a,   127-k)
                                    out col a*128 + 2k+1 = (a+1, 127-k).
    Single tiles keep plain layout."""
    T = cfg.T
    src_col = np.empty(T * 128, np.int64)
    for w in range(cfg.W):
        for ty in range(2):
            base = int(cfg.gcol[w, ty])
            nov = int(cfg.ov[w, ty])
            pairs = [(base + t, base + t + 1) for t in range(0, KI, 2)]
            pairs += [(base + KI + t, base + KI + t + 1)
                      for t in range(0, nov - 1, 2)]
            singles = ([base + KI + nov - 1] if nov % 2 else [])
            for (a, b) in pairs:
                k = np.arange(128)
                src_col[a * 128 + 2 * k] = a * 128 + (127 - k)
                src_col[a * 128 + 2 * k + 1] = b * 128 + (127 - k)
            for s in singles:
                src_col[s * 128:(s + 1) * 128] = np.arange(
                    s * 128, (s + 1) * 128)
    return src_col


def prep_inputs(cfg: Cfg, x, edge_index, edge_type, w_sl, b_sl,
                w1_n, b1_n, gamma_n, beta_n, w2_n, b2_n,
                w1_d, b1_d, gamma_d, beta_d, w2_d, b2_d,
                w_gat, b_gat):
    C, npc, npad = cfg.C, cfg.npc, cfg.npad
    x = np.asarray(x, np.float32)
    src = np.asarray(edge_index[0]).astype(np.int64)
    dst = np.asarray(edge_index[1]).astype(np.int64)
    et = np.asarray(edge_type).astype(np.int64)

    off, sel = pack_edges(cfg, src, dst, et)

    xf8 = np.vstack([x, np.zeros((1, 128), np.float32)]).astype(F8)
    perm = drswi_maps(cfg) if MSG_DRSWI else None
    streams = []
    for c in range(C):
        m = np.ascontiguousarray(xf8[off[c]].reshape(128, -1))
        if perm is not None:
            m = np.ascontiguousarray(m[:, perm])
        streams.append(m)

    xTs = []
    for c in range(C):
        xp = np.zeros((npad, 128), np.float32)
        xp[:npc] = x[c * npc:(c + 1) * npc]
        xTs.append(np.ascontiguousarray(xp.T).astype(BF))

    def bt(a):
        return np.ascontiguousarray(np.asarray(a, np.float64)).astype(BF)

    w_sl64 = np.asarray(w_sl, np.float64)
    w2n64 = np.asarray(w2_n, np.float64)
    w2d64 = np.asarray(w2_d, np.float64)
    wg = np.asarray(w_gat, np.float64)
    wg0, wg1, wg2 = wg[:, 0:128], wg[:, 128:256], wg[:, 256:384]

    wcols = [
        bt(w_sl64.T), bt(np.asarray(w1_n).T), bt(np.asarray(w1_d).T),
        bt(w2n64.T), bt(w2d64[::-1, :].T),
        bt((wg0 @ w_sl64).T), bt((wg1 @ w2n64).T), bt((wg2 @ w2d64).T),
        bt(np.triu(np.ones((128, 128), np.float32))),
    ]
    wpack = np.concatenate(wcols, axis=1)

    bias_at = (np.asarray(b_sl, np.float64)
               + np.asarray(b2_n, np.float64))
    bias_xd = np.asarray(b2_d, np.float64)[::-1]
    rows = np.concatenate([bias_at, bias_xd, bias_at, bias_xd]
                          )[None, :].astype(BF)

    bz = (np.asarray(b_gat, np.float64) + wg0 @ np.asarray(b_sl, np.float64)
          + wg1 @ np.asarray(b2_n, np.float64)
          + wg2 @ np.asarray(b2_d, np.float64))
    g_n = np.asarray(gamma_n, np.float64)
    g_d = np.asarray(gamma_d, np.float64)
    vecs = np.stack([
        g_n, np.asarray(beta_n, np.float64) / g_n,
        g_d, np.asarray(beta_d, np.float64) / g_d,
        bz,
    ], axis=1).astype(np.float32)

    in_maps = []
    for c in range(C):
        m = {
            "stream": streams[c],
            "xT": xTs[c],
            "sel": np.ascontiguousarray(sel[c]),
            "wpack": wpack,
            "rows": rows,
            "vecs": vecs,
            "iota128": np.broadcast_to(
                np.arange(128, dtype=np.float32)[None, :],
                (128, 128)).copy(),
            "ipair": np.concatenate(
                [np.eye(128, dtype=np.float32)] * 2, axis=1).astype(F8),
        }
        in_maps.append(m)
    return in_maps


_BUILD_CACHE = {}


def make_cfg(inputs):
    dst = np.asarray(inputs["edge_index"][1]).astype(np.int64)
    et = np.asarray(inputs["edge_type"]).astype(np.int64)
    sched = build_schedule(dst, et, N_GLOBAL, CORES)
    return Cfg(N_GLOBAL, E_GLOBAL, CORES, sched)


def run(cfg: Cfg, inputs: dict, **run_kwargs):
    in_maps = prep_inputs(cfg, **inputs)
    key = (cfg.N, cfg.E, cfg.C, MSG_DRSWI, cfg.sched)
    if key not in _BUILD_CACHE:
        _BUILD_CACHE[key] = build(cfg)
    nc = _BUILD_CACHE[key]
    res = run_bass_kernel_spmd(nc, in_maps, core_ids=list(range(cfg.C)),
                               **run_kwargs)
    outs = [res.results[c]["out"][:cfg.npc] for c in range(cfg.C)]
    return np.concatenate(outs, axis=0).astype(np.float32), res


def kernel(**inputs):
    out, _ = run(make_cfg(inputs), inputs)
    return out
